# revision 6
# baseline (speedup 1.0000x reference)
"""DisparityWarp Trainium2 kernel (Bass/Tile) — v3.

Contract: kernel(src, disparity) takes FULL inputs
  src [8, 32, 384, 768] f32, disparity [8, 1, 384, 768] f32
and returns the FULL output [8, 32, 384, 768] f32 of
  grid_sample(src, grid, bilinear, zeros, align_corners=False)
with grid_x = 2*(xx - disp)/(W-1) - 1, grid_y = 2*yy/(H-1) - 1.

Sharding: pure data parallel, one batch per NeuronCore (8 cores).

Per-core algorithm: unnormalized coords ix = (x - d)*W/(W-1) - 0.5,
iy = y*H/(H-1) - 0.5. Vertical lerp weights depend only on y; the
horizontal warp is banded: out[c,x] = sum_x' vrow[c,x'] * hat(ix[x]-x')
with hat(u) = max(0, 1-|u|), and x' restricted to [0, W) (zero pad).

Geometry: output cols in blocks of BLK=94; window j covers
x' in [lo_j, hi_j), lo_j = max(94j-33, 0), hi_j = min(94j+95, W).

Per 3-row group g (quad base qb = clamp(floor(iy[3g]), 0, H-4)):
  s4   [128=(4r,32c), W] f16  <- gpsimd casting DMA of src[:, qb:qb+4, :]
  D:   dp_i = ld3.T @ [ones;int;frac] = ixm1[x] - (p+1), emitted FIRST
       so the evac engines start early.
  VT:  vtp[x'loc, (3i,32c)] = s4[:, lo_j:hi_j].T @ v2neg[g]  (9 matmuls)
  vtx  [128, 864] f16 <- vtp (ACT takes cols 0:480, DVE the rest)
  weight chain per row i: asb = |dp_i| computed in TWO engine-parallel
       halves (ACT cols 0:384, DVE 384:768; DVE uses abs_max) so the
       dp PSUM buffer frees at max(engine) latency, not their sum;
       wsb_i = min(asb-1, 0) == -hat, one DVE 4x f16 op per row.
  gathers: outp[32i:+32, blk_j] = vtx[.., win_j, i].T @ wsb_i[:, blk_j]
       (tile_position (0,32i); window 8 contracts K=49 only)
  outp evac is split by PSUM bank: bank0 (blocks 0-4) evacs on DVE as
       soon as j=4's gathers land; bank1 (blocks 5-8) on ACT after j=8.

The PE stream runs gathers one group behind VT/D so the weight chain
(ACT/DVE) hides behind the next group's PE work.  DMA discipline (the
real bottleneck on this stack): prefetch DMAs (s4 via Pool/SWDGE with
f32->f16 cast; slab staging via SP) are issued 4-16 groups ahead and
never wait; output stores are batched 8 groups per DMA set, buffered
through a wide SBUF tile, and drained 1-2 per iteration with a
backlog so their waits are resolved at issue.
"""

import sys

if "/opt/trn_rl_repo" not in sys.path:
    sys.path.insert(0, "/opt/trn_rl_repo")

from contextlib import ExitStack

import numpy as np

import concourse.bass as bass
import concourse.mybir as mybir
from concourse import bacc
from concourse.tile import TileContext

F32 = mybir.dt.float32
F16 = mybir.dt.float16
I32 = mybir.dt.int32
AF = mybir.ActivationFunctionType
ALU = mybir.AluOpType

B, C, H, W = 8, 32, 384, 768
S = W / (W - 1)
BLK = 94           # output columns per block
NB = 9             # ceil(W / BLK)
GRP = 3            # output rows per group
NG = H // GRP      # 128 groups
N_CORES = 8

WIN_LO = [max(BLK * j - 33, 0) for j in range(NB)]
WIN_HI = [min(lo + 128, W) for lo in WIN_LO]
# vtp / outp PSUM column offsets (bank-aligned: banks of 512 f32)
VCOL = [96 * j if j < 5 else 512 + 96 * (j - 5) for j in range(NB)]
OCOL = [BLK * j if j < 5 else 512 + BLK * (j - 5) for j in range(NB)]
# merged vt SBUF tile column offsets (no bank constraint in SBUF)
XCOL = [96 * j for j in range(NB)]
HALF = 384         # asb engine-split point


# ---------------------------------------------------------------- constants
def _vert_coefs():
    yy = np.arange(H, dtype=np.float64)
    iy = yy * (H / (H - 1)) - 0.5
    y0 = np.floor(iy).astype(np.int64)
    fy = iy - y0
    a = (1.0 - fy) * ((y0 >= 0) & (y0 < H))
    b = fy * ((y0 + 1 >= 0) & (y0 + 1 < H))
    return a, b, y0


def _host_constants():
    a, b, y0 = _vert_coefs()
    # V2NEG [128 p=(4r,32c), NG, 96 m=(3i,32c)] fp16, negated blend coefs
    v2 = np.zeros((4, C, NG, GRP, C), dtype=np.float32)
    quad_bases = []
    for g in range(NG):
        ys = [GRP * g + i for i in range(GRP)]
        qbase = min(max(int(y0[ys[0]]), 0), H - 4)
        quad_bases.append(qbase)
        for i, y in enumerate(ys):
            ra = int(y0[y]) - qbase
            rb = ra + 1
            for c in range(C):
                if a[y] != 0.0:
                    assert 0 <= ra <= 3
                    v2[ra, c, g, i, c] += -a[y]
                if b[y] != 0.0:
                    assert 0 <= rb <= 3
                    v2[rb, c, g, i, c] += -b[y]
    v2neg = v2.reshape(4 * C, NG, GRP * C).astype(np.float16)

    # D matmul stationary: rows [-(p+1), 1, 1]; rhs rows [ones, int, frac]
    ld3 = np.stack([
        -(np.arange(128, dtype=np.float32) + 1.0),
        np.ones(128, dtype=np.float32),
        np.ones(128, dtype=np.float32),
    ]).astype(np.float16)                                        # [3, 128]
    onesw = np.ones((1, 2 * GRP, W), dtype=np.float16)           # [1, 6, W]

    x = np.arange(W, dtype=np.float64)
    base = np.array([WIN_LO[int(xi) // BLK] for xi in x], dtype=np.float64)
    cf = (x * S - 0.5 - base + 1.0).astype(np.float32)[None, :]  # [1, W]
    return v2neg, ld3, onesw, cf, quad_bases


# ---------------------------------------------------------------- program
def build_nc(ngroups=NG, repeat=1):
    _, _, _, _, quad_bases = _host_constants()  # (v2neg, ld3, onesw, cf, qb)
    nc = bacc.Bacc("TRN2", target_bir_lowering=False, debug=False)

    src = nc.dram_tensor("src", [C, H, W], F32, kind="ExternalInput").ap()
    disp = nc.dram_tensor("disp", [H, W], F32, kind="ExternalInput").ap()
    v2d = nc.dram_tensor("v2neg", [4 * C, NG, GRP * C], F16,
                         kind="ExternalInput").ap()
    ld3d = nc.dram_tensor("ld3", [3, 128], F16, kind="ExternalInput").ap()
    onesd = nc.dram_tensor("onesw", [1, 2 * GRP, W], F16,
                           kind="ExternalInput").ap()
    cfd = nc.dram_tensor("cf", [1, W], F32, kind="ExternalInput").ap()
    outd = nc.dram_tensor("out", [C, H, W], F32, kind="ExternalOutput").ap()

    ngr = min(ngroups, NG)
    nrows = GRP * ngr
    nt = (nrows + 127) // 128

    with ExitStack() as ctx:
        tc = ctx.enter_context(TileContext(nc))
        singles = ctx.enter_context(tc.tile_pool(name="singles", bufs=1))
        ph1 = ctx.enter_context(tc.tile_pool(name="ph1", bufs=1))
        s4p = ctx.enter_context(tc.tile_pool(name="s4p", bufs=6))
        vtxp = ctx.enter_context(tc.tile_pool(name="vtxp", bufs=2))
        asbp = ctx.enter_context(tc.tile_pool(name="asbp", bufs=3))
        wp = ctx.enter_context(tc.tile_pool(name="wp", bufs=6))
        obufp = ctx.enter_context(tc.tile_pool(name="obufp", bufs=2))
        vtpp = ctx.enter_context(tc.tile_pool(name="vtpp", bufs=1, space="PSUM"))
        dpp = ctx.enter_context(tc.tile_pool(name="dpp", bufs=2, space="PSUM"))
        outpp = ctx.enter_context(tc.tile_pool(name="outpp", bufs=1, space="PSUM"))

        def emit_body():
            # ---- constants ----
            v2sb = singles.tile([4 * C, NG, GRP * C], F16)
            nc.sync.dma_start(out=v2sb, in_=v2d)
            # ld3 replicated at bases 0/32/64 to match the slab ring's
            # base partition (matmul operands must share base_partition)
            ld3rt = singles.tile([96, 128], F16, name="ld3rt", tag="ld3rt")
            for k in range(3):
                nc.sync.dma_start(out=ld3rt[32 * k:32 * k + 3], in_=ld3d)
            cfb = singles.tile([128, W], F32)
            nc.sync.dma_start(out=cfb, in_=cfd.to_broadcast((128, W)))

            # ---- persistent slab ring, packed on partitions (3 per
            # entry) so the ring costs one column range, not NSLAB ----
            NSLAB = 3
            SLABG = 4          # groups per slab
            slabrt = singles.tile([96, SLABG * GRP, W], F16,
                                  name="slabrt", tag="slabrt")
            slabring = [slabrt[32 * k:32 * k + 3] for k in range(NSLAB)]
            for t_ in slabring:
                nc.vector.memset(t_[0:1, :, :], 1.0)

            # ---- staging helpers ----
            def issue_s4(g):
                qb = quad_bases[g]
                s4 = s4p.tile([128, W], F16, name=f"s4_{g}", tag="s4")
                in_ap = src[:, qb:qb + 4, :].rearrange("c r x -> r c x")
                nc.gpsimd.dma_start(out=s4, in_=in_ap)
                return s4

            # ---- prologue: source prefetches first (no deps) ----
            nslabs = (ngr + SLABG - 1) // SLABG
            pre_s4 = {}
            pre_slab = {}
            for g in range(min(5, ngr)):
                pre_s4[g] = issue_s4(g)

            # ---- phase 1: disparity -> int/frac fp16 [128, nt, W] ----
            int16 = singles.tile([128, nt, W], F16)
            frac16 = singles.tile([128, nt, W], F16)

            def phase1_tile(t):
                r0 = 128 * t
                nr = min(128, H - r0)
                dt_ = ph1.tile([128, W], F32)
                nc.sync.dma_start(out=dt_[:nr], in_=disp[r0:r0 + nr, :])
                ixm1 = ph1.tile([128, W], F32)
                nc.vector.tensor_scalar_mul(ixm1[:nr], dt_[:nr], -float(S))
                nc.vector.tensor_add(ixm1[:nr], ixm1[:nr], cfb[:nr])
                iv = ph1.tile([128, W], I32)
                nc.vector.tensor_copy(iv[:nr], ixm1[:nr])
                fv = ph1.tile([128, W], F32)
                nc.vector.tensor_copy(fv[:nr], iv[:nr])
                nc.scalar.copy(int16[:nr, t, :], fv[:nr])
                fr = ph1.tile([128, W], F32)
                nc.vector.tensor_sub(fr[:nr], ixm1[:nr], fv[:nr])
                nc.scalar.copy(frac16[:nr, t, :], fr[:nr])

            def issue_slab(sidx):
                # One slab serves groups [SLABG*sidx, SLABG*(sidx+1)).
                # slab partitions: 0 = ones (persistent), 1 = int, 2 = frac.
                y0 = SLABG * GRP * sidx
                cnt = min(SLABG * GRP, nrows - y0)
                slab = slabring[sidx % NSLAB]
                for part, tsrc in ((1, int16), (2, frac16)):
                    done = 0
                    while done < cnt:
                        y = y0 + done
                        p, t = y % 128, y // 128
                        n = min(cnt - done, 128 - p)
                        nc.sync.dma_start(
                            out=slab[part:part + 1, done:done + n, :],
                            in_=tsrc[p:p + n, t, :])
                        done += n
                return slab

            # phase-1 tile 0 unlocks the first two slabs; later tiles
            # are processed after the prologue prefetches are queued.
            phase1_tile(0)
            for p_ in range(min(2, nslabs)):
                pre_slab[p_] = issue_slab(p_)
            for t in range(1, nt):
                phase1_tile(t)

            def gathers_one(rec, i, j, outp):
                wsb = rec["wsb"][i]
                vtx = rec["vtx"]
                n = min(BLK, W - BLK * j)
                if j < 8:
                    lhs = vtx[:, XCOL[j] + 32 * i:XCOL[j] + 32 * i + 32]
                    rhs = wsb[:, BLK * j:BLK * j + n]
                else:
                    lhs = vtx[0:49, XCOL[8] + 32 * i:XCOL[8] + 32 * i + 32]
                    rhs = wsb[0:49, BLK * j:BLK * j + n]
                nc.tensor.matmul(
                    outp[32 * i:32 * i + 32, OCOL[j]:OCOL[j] + n],
                    lhs, rhs, start=True, stop=True,
                    tile_position=(0, 32 * i),
                )

            # Output stores are batched OBATCH groups per DMA: per-group PSUM
            # evac copies land in one wide SBUF tile; a single DMA (emitted
            # right after the batch's last copy, so its wait is ~resolved)
            # stores 3*OBATCH rows.  Few out DMAs -> the 8 round-robin DMAHW
            # completion lanes are never blocked by long-waiting stores, which
            # otherwise throttles the prefetch DMA stream behind them.
            OBATCH = 8
            obuf_state = {"tile": None, "base": -1}
            pending_out = []   # queued store DMAs, drained 1-2 per iteration

            def obuf_for(pg):
                b = pg % OBATCH
                if b == 0:
                    obuf_state["tile"] = obufp.tile([96, OBATCH, W], F32,
                                                    name=f"obuf_{pg}",
                                                    tag="obuf")
                    obuf_state["base"] = pg
                return obuf_state["tile"], b

            def store_flush(prev):
                pg = prev["g"]
                b = pg % OBATCH
                if b == OBATCH - 1 or pg == ngr - 1:
                    g0 = obuf_state["base"]
                    nb = pg - g0 + 1
                    obuf = obuf_state["tile"]
                    rows = outd[:, GRP * g0:GRP * (g0 + nb), :]
                    for i in range(GRP):
                        pending_out.append(
                            (rows[:, i::GRP, :], obuf[32 * i:32 * i + 32, 0:nb, :]))

            def drain_out(all_=False):
                # 1-2 store DMAs per iteration: data is >=1 batch old, so the
                # wait is resolved at issue and the SP queue never head-blocks.
                if all_:
                    n = len(pending_out)
                else:
                    n = min(len(pending_out), 2 if len(pending_out) > 3 else 1)
                for _ in range(n):
                    o, i_ = pending_out.pop(0)
                    nc.sync.dma_start(out=o, in_=i_)

            prev = None
            for g in range(ngr):
                s4 = pre_s4.pop(g)
                slab = pre_slab[g // SLABG]
                ld3sb = ld3rt[32 * ((g // SLABG) % 3):32 * ((g // SLABG) % 3) + 3]
                if g % SLABG == SLABG - 1:
                    del pre_slab[g // SLABG]
                iloc0 = GRP * (g % SLABG)

                # ---- prefetch first: these SP DMAs have no unresolved waits,
                # so they must sit AHEAD of the out DMAs in the SP queue ----
                if g + 5 < ngr:
                    pre_s4[g + 5] = issue_s4(g + 5)
                if g % SLABG == 0 and g // SLABG + 2 < nslabs:
                    pre_slab[g // SLABG + 2] = issue_slab(g // SLABG + 2)
                drain_out()

                # ---- D rows 0,1 FIRST: unblocks the ACT/DVE weight chain
                # at iteration start so it runs under the PE stream ----
                dp0 = dpp.tile([128, 1024], F32, name=f"dp0_{g}", tag="dp")
                nc.tensor.matmul(dp0[:, 0:512], ld3sb,
                                 slab[0:3, iloc0 + 0, 0:512],
                                 start=True, stop=True)
                nc.tensor.matmul(dp0[:, 512:768], ld3sb,
                                 slab[0:3, iloc0 + 0, 512:W],
                                 start=True, stop=True)
                dp1 = dpp.tile([128, 1024], F32, name=f"dp1_{g}", tag="dp")
                nc.tensor.matmul(dp1[:, 0:512], ld3sb,
                                 slab[0:3, iloc0 + 1, 0:512],
                                 start=True, stop=True)
                nc.tensor.matmul(dp1[:, 512:768], ld3sb,
                                 slab[0:3, iloc0 + 1, 512:W],
                                 start=True, stop=True)

                # weight chain rows 0,1 abs (ACT)
                asb0 = asbp.tile([128, W], F16, name=f"asb0_{g}", tag="asb")
                nc.scalar.activation(asb0, dp0[:, 0:W], AF.Abs)
                asb1 = asbp.tile([128, W], F16, name=f"asb1_{g}", tag="asb")
                nc.scalar.activation(asb1, dp1[:, 0:W], AF.Abs)

                # ---- VT: blend-transpose, 9 windows ----
                vtp = vtpp.tile([128, 1024], F32, name=f"vtp_{g}", tag="vtp")
                for j in range(NB):
                    m = WIN_HI[j] - WIN_LO[j]
                    nc.tensor.matmul(
                        vtp[0:m, VCOL[j]:VCOL[j] + GRP * C],
                        s4[:, WIN_LO[j]:WIN_HI[j]],
                        v2sb[:, g, :],
                        start=True, stop=True,
                    )
                # merged vt evac FIRST in the DVE queue (latency-critical:
                # gates the next group's VT; wsb has a full group of slack)
                vtx = vtxp.tile([128, 864], F16, name=f"vtx_{g}", tag="vtx")
                nc.vector.tensor_copy(vtx[:, 0:480], vtp[:, 0:480])
                nc.vector.tensor_copy(vtx[:, 480:768], vtp[:, 512:800])
                nc.vector.tensor_copy(vtx[0:49, 768:864], vtp[0:49, 800:896])

                wsb0 = wp.tile([128, W], F16, name=f"wsb0_{g}", tag="wsb")
                nc.vector.tensor_scalar(out=wsb0, in0=asb0, scalar1=1.0,
                                        scalar2=0.0, op0=ALU.subtract,
                                        op1=ALU.min)
                wsb1 = wp.tile([128, W], F16, name=f"wsb1_{g}", tag="wsb")
                nc.vector.tensor_scalar(out=wsb1, in0=asb1, scalar1=1.0,
                                        scalar2=0.0, op0=ALU.subtract,
                                        op1=ALU.min)

                # ---- D row 2 + chain ----
                dp2 = dpp.tile([128, 1024], F32, name=f"dp2_{g}", tag="dp")
                nc.tensor.matmul(dp2[:, 0:512], ld3sb,
                                 slab[0:3, iloc0 + 2, 0:512],
                                 start=True, stop=True)
                nc.tensor.matmul(dp2[:, 512:768], ld3sb,
                                 slab[0:3, iloc0 + 2, 512:W],
                                 start=True, stop=True)
                asb2 = asbp.tile([128, W], F16, name=f"asb2_{g}", tag="asb")
                nc.scalar.activation(asb2, dp2[:, 0:W], AF.Abs)
                wsb2 = wp.tile([128, W], F16, name=f"wsb2_{g}", tag="wsb")
                nc.vector.tensor_scalar(out=wsb2, in0=asb2, scalar1=1.0,
                                        scalar2=0.0, op0=ALU.subtract,
                                        op1=ALU.min)

                # ---- gathers for previous group ----
                if prev is not None:
                    outp = outpp.tile([96, 1024], F32, name=f"outp_{g}",
                                      tag="outp")
                    obuf, b = obuf_for(prev["g"])
                    # j-major, i inner: consecutive MMs target different
                    # 32-wide PE column tiles, so LDW/streaming overlap
                    # across sub-arrays on real hardware
                    for j_ in range(5):
                        for i_ in range(GRP):
                            gathers_one(prev, i_, j_, outp)
                    # bank0 evac as soon as its gathers land (DVE)
                    nc.vector.tensor_copy(obuf[:, b, 0:470], outp[:, 0:470])
                    for j_ in range(5, NB):
                        for i_ in range(GRP):
                            gathers_one(prev, i_, j_, outp)
                    nc.scalar.copy(obuf[:, b, 470:W], outp[:, 512:810])
                    store_flush(prev)

                prev = {"g": g, "wsb": (wsb0, wsb1, wsb2), "vtx": vtx}

            # ---- epilogue: drain last group ----
            outp = outpp.tile([96, 1024], F32, name="outp_last", tag="outp")
            obuf, b = obuf_for(prev["g"])
            for j_ in range(5):
                for i_ in range(GRP):
                    gathers_one(prev, i_, j_, outp)
            nc.vector.tensor_copy(obuf[:, b, 0:470], outp[:, 0:470])
            for j_ in range(5, NB):
                for i_ in range(GRP):
                    gathers_one(prev, i_, j_, outp)
            nc.scalar.copy(obuf[:, b, 470:W], outp[:, 512:810])
            store_flush(prev)
            drain_out(all_=True)

        if repeat > 1:
            with tc.For_i(0, repeat):
                emit_body()
        else:
            emit_body()


    nc.finalize()
    return nc


_NC_CACHE = {}


def _get_nc(ngroups=NG):
    if ngroups not in _NC_CACHE:
        _NC_CACHE[ngroups] = build_nc(ngroups)
    return _NC_CACHE[ngroups]


# ---------------------------------------------------------------- entry
def kernel(src: np.ndarray, disparity: np.ndarray) -> np.ndarray:
    from concourse.bass_utils import run_bass_kernel_spmd

    src = np.ascontiguousarray(np.asarray(src), dtype=np.float32)
    disparity = np.ascontiguousarray(np.asarray(disparity), dtype=np.float32)
    v2neg, ld3, onesw, cf, _ = _host_constants()
    nc = _get_nc()
    in_maps = []
    for b in range(B):
        in_maps.append({
            "src": src[b],
            "disp": disparity[b, 0],
            "v2neg": v2neg,
            "ld3": ld3,
            "onesw": onesw,
            "cf": cf,
        })
    res = run_bass_kernel_spmd(nc, in_maps, core_ids=list(range(N_CORES)))
    out = np.stack([res.results[b]["out"] for b in range(B)])
    return out.astype(np.float32)


# ---------------------------------------------------------------- sim test
def _np_reference(src, disp):
    """Single-core numpy reference (mirror of reference.py)."""
    Cc, Hh, Ww = src.shape
    xx = np.arange(Ww, dtype=np.float32)
    ix = (xx[None, :] - disp) * (Ww / (Ww - 1)) - 0.5          # [H, W]
    yy = np.arange(Hh, dtype=np.float32)
    iy = np.broadcast_to((yy * (Hh / (Hh - 1)) - 0.5)[:, None], (Hh, Ww))
    x0 = np.floor(ix).astype(np.int64)
    y0 = np.floor(iy).astype(np.int64)
    fx = ix - x0
    fy = iy - y0

    def gather(yi, xi):
        inb = ((yi >= 0) & (yi < Hh) & (xi >= 0) & (xi < Ww))
        yc = np.clip(yi, 0, Hh - 1)
        xc = np.clip(xi, 0, Ww - 1)
        v = src[:, yc, xc]                                      # [C, H, W]
        return v * inb[None]

    w00 = (1 - fy) * (1 - fx)
    w01 = (1 - fy) * fx
    w10 = fy * (1 - fx)
    w11 = fy * fx
    return (gather(y0, x0) * w00 + gather(y0, x0 + 1) * w01 +
            gather(y0 + 1, x0) * w10 + gather(y0 + 1, x0 + 1) * w11)


def _sim_check(ngroups=2):
    from concourse.bass_interp import CoreSim

    rng = np.random.default_rng(0)
    src = rng.standard_normal((C, H, W)).astype(np.float32)
    disp = (rng.random((H, W)) * 32.0).astype(np.float32)
    v2neg, ld3, onesw, cf, _ = _host_constants()

    nc = build_nc(ngroups)
    sim = CoreSim(nc)
    for name, val in (("src", src), ("disp", disp), ("v2neg", v2neg),
                      ("ld3", ld3), ("onesw", onesw), ("cf", cf)):
        sim.tensor(name)[:] = val
    sim.simulate(check_with_hw=False)
    got = np.array(sim.tensor("out"))

    ref = _np_reference(src, disp)
    ys = slice(0, GRP * ngroups)
    diff = got[:, ys] - ref[:, ys]
    rel = np.linalg.norm(diff) / np.linalg.norm(ref[:, ys])
    print(f"sim rows[0:{GRP * ngroups}]  max abs "
          f"{np.abs(diff).max():.3e}  rel l2 {rel:.3e}")
    return rel


if __name__ == "__main__":
    ng = int(sys.argv[1]) if len(sys.argv) > 1 else 2
    _sim_check(ng)


# revision 9
# speedup vs baseline: 1.1152x; 1.1152x over previous
"""DisparityWarp Trainium2 kernel (Bass/Tile) — v3.

Contract: kernel(src, disparity) takes FULL inputs
  src [8, 32, 384, 768] f32, disparity [8, 1, 384, 768] f32
and returns the FULL output [8, 32, 384, 768] f32 of
  grid_sample(src, grid, bilinear, zeros, align_corners=False)
with grid_x = 2*(xx - disp)/(W-1) - 1, grid_y = 2*yy/(H-1) - 1.

Sharding: pure data parallel, one batch per NeuronCore (8 cores).

Per-core algorithm: unnormalized coords ix = (x - d)*W/(W-1) - 0.5,
iy = y*H/(H-1) - 0.5. Vertical lerp weights depend only on y; the
horizontal warp is banded: out[c,x] = sum_x' vrow[c,x'] * hat(ix[x]-x')
with hat(u) = max(0, 1-|u|), and x' restricted to [0, W) (zero pad).

Geometry: output cols in blocks of BLK=94; window j covers
x' in [lo_j, hi_j), lo_j = max(94j-33, 0), hi_j = min(94j+95, W).

Per 3-row group g (quad base qb = clamp(floor(iy[3g]), 0, H-4)):
  s4   [128=(4r,32c), W] f16  <- gpsimd casting DMA of src[:, qb:qb+4, :]
  D:   dp_i = ld3.T @ [ones;int;frac] = ixm1[x] - (p+1), emitted FIRST
       so the evac engines start early.
  VT:  vtp[x'loc, (3i,32c)] = s4[:, lo_j:hi_j].T @ v2neg[g]  (9 matmuls)
  vtx  [128, 864] f16 <- vtp (ACT takes cols 0:480, DVE the rest)
  weight chain per row i: asb = |dp_i| computed in TWO engine-parallel
       halves (ACT cols 0:384, DVE 384:768; DVE uses abs_max) so the
       dp PSUM buffer frees at max(engine) latency, not their sum;
       wsb_i = min(asb-1, 0) == -hat, one DVE 4x f16 op per row.
  gathers: outp[32i:+32, blk_j] = vtx[.., win_j, i].T @ wsb_i[:, blk_j]
       (tile_position (0,32i); window 8 contracts K=49 only)
  outp evac is split by PSUM bank: bank0 (blocks 0-4) evacs on DVE as
       soon as j=4's gathers land; bank1 (blocks 5-8) on ACT after j=8.

The PE stream runs gathers one group behind VT/D so the weight chain
(ACT/DVE) hides behind the next group's PE work.  DMA discipline (the
real bottleneck on this stack): prefetch DMAs (s4 via Pool/SWDGE with
f32->f16 cast; slab staging via SP) are issued 4-16 groups ahead and
never wait; output stores are batched 8 groups per DMA set, buffered
through a wide SBUF tile, and drained 1-2 per iteration with a
backlog so their waits are resolved at issue.
"""

import sys

if "/opt/trn_rl_repo" not in sys.path:
    sys.path.insert(0, "/opt/trn_rl_repo")

from contextlib import ExitStack

import numpy as np

import concourse.bass as bass
import concourse.mybir as mybir
from concourse import bacc
from concourse.tile import TileContext

F32 = mybir.dt.float32
F16 = mybir.dt.float16
I32 = mybir.dt.int32
AF = mybir.ActivationFunctionType
ALU = mybir.AluOpType

B, C, H, W = 8, 32, 384, 768
S = W / (W - 1)
BLK = 94           # output columns per block
NB = 9             # ceil(W / BLK)
GRP = 3            # output rows per group
NG = H // GRP      # 128 groups
N_CORES = 8

WIN_LO = [max(min(BLK * j - 33, W - 128), 0) for j in range(NB)]
WIN_HI = [lo + 128 for lo in WIN_LO]
# vtp / outp PSUM column offsets (bank-aligned: banks of 512 f32)
VCOL = [96 * j if j < 5 else 512 + 96 * (j - 5) for j in range(NB)]
OCOL = [BLK * j if j < 5 else 512 + BLK * (j - 5) for j in range(NB)]
# merged vt SBUF tile column offsets (no bank constraint in SBUF)
XCOL = [96 * j for j in range(NB)]
HALF = 384         # asb engine-split point


# ---------------------------------------------------------------- constants
def _vert_coefs():
    yy = np.arange(H, dtype=np.float64)
    iy = yy * (H / (H - 1)) - 0.5
    y0 = np.floor(iy).astype(np.int64)
    fy = iy - y0
    a = (1.0 - fy) * ((y0 >= 0) & (y0 < H))
    b = fy * ((y0 + 1 >= 0) & (y0 + 1 < H))
    return a, b, y0


def _host_constants():
    a, b, y0 = _vert_coefs()
    # V2NEG [128 p=(4r,32c), NG, 96 m=(3i,32c)] fp16, negated blend coefs
    v2 = np.zeros((4, C, NG, GRP, C), dtype=np.float32)
    quad_bases = []
    for g in range(NG):
        ys = [GRP * g + i for i in range(GRP)]
        qbase = min(max(int(y0[ys[0]]), 0), H - 4)
        quad_bases.append(qbase)
        for i, y in enumerate(ys):
            ra = int(y0[y]) - qbase
            rb = ra + 1
            for c in range(C):
                if a[y] != 0.0:
                    assert 0 <= ra <= 3
                    v2[ra, c, g, i, c] += -a[y]
                if b[y] != 0.0:
                    assert 0 <= rb <= 3
                    v2[rb, c, g, i, c] += -b[y]
    v2neg = v2.reshape(4 * C, NG, GRP * C).astype(np.float16)

    # D matmul stationary: rows [-(p+1), 1, 1]; rhs rows [ones, int, frac]
    ld3 = np.stack([
        -(np.arange(128, dtype=np.float32) + 1.0),
        np.ones(128, dtype=np.float32),
        np.ones(128, dtype=np.float32),
    ]).astype(np.float16)                                        # [3, 128]
    onesw = np.ones((1, 2 * GRP, W), dtype=np.float16)           # [1, 6, W]

    x = np.arange(W, dtype=np.float64)
    base = np.array([WIN_LO[int(xi) // BLK] for xi in x], dtype=np.float64)
    cf = (x * S - 0.5 - base + 1.0).astype(np.float32)[None, :]  # [1, W]
    return v2neg, ld3, onesw, cf, quad_bases


# ---------------------------------------------------------------- program
def build_nc(ngroups=NG, repeat=1):
    _, _, _, _, quad_bases = _host_constants()  # (v2neg, ld3, onesw, cf, qb)
    nc = bacc.Bacc("TRN2", target_bir_lowering=False, debug=False)

    src = nc.dram_tensor("src", [C, H, W], F32, kind="ExternalInput").ap()
    disp = nc.dram_tensor("disp", [H, W], F32, kind="ExternalInput").ap()
    v2d = nc.dram_tensor("v2neg", [4 * C, NG, GRP * C], F16,
                         kind="ExternalInput").ap()
    ld3d = nc.dram_tensor("ld3", [3, 128], F16, kind="ExternalInput").ap()
    onesd = nc.dram_tensor("onesw", [1, 2 * GRP, W], F16,
                           kind="ExternalInput").ap()
    cfd = nc.dram_tensor("cf", [1, W], F32, kind="ExternalInput").ap()
    outd = nc.dram_tensor("out", [C, H, W], F32, kind="ExternalOutput").ap()

    ngr = min(ngroups, NG)
    nrows = GRP * ngr
    nt = (nrows + 127) // 128

    with ExitStack() as ctx:
        tc = ctx.enter_context(TileContext(nc))
        singles = ctx.enter_context(tc.tile_pool(name="singles", bufs=1))
        ph1 = ctx.enter_context(tc.tile_pool(name="ph1", bufs=1))
        s4p = ctx.enter_context(tc.tile_pool(name="s4p", bufs=6))
        vtxp = ctx.enter_context(tc.tile_pool(name="vtxp", bufs=2))
        asbp = ctx.enter_context(tc.tile_pool(name="asbp", bufs=3))
        wp = ctx.enter_context(tc.tile_pool(name="wp", bufs=6))
        obufp = ctx.enter_context(tc.tile_pool(name="obufp", bufs=2))
        vtpp = ctx.enter_context(tc.tile_pool(name="vtpp", bufs=1, space="PSUM"))
        dpp = ctx.enter_context(tc.tile_pool(name="dpp", bufs=2, space="PSUM"))
        outpp = ctx.enter_context(tc.tile_pool(name="outpp", bufs=1, space="PSUM"))

        def emit_body():
            # ---- constants ----
            v2sb = singles.tile([4 * C, NG, GRP * C], F16)
            nc.sync.dma_start(out=v2sb, in_=v2d)
            # ld3 replicated at bases 0/32/64 to match the slab ring's
            # base partition (matmul operands must share base_partition)
            ld3rt = singles.tile([96, 128], F16, name="ld3rt", tag="ld3rt")
            for k in range(3):
                nc.sync.dma_start(out=ld3rt[32 * k:32 * k + 3], in_=ld3d)
            cfb = singles.tile([128, W], F32)
            nc.sync.dma_start(out=cfb, in_=cfd.to_broadcast((128, W)))

            # ---- persistent slab ring, packed on partitions (3 per
            # entry) so the ring costs one column range, not NSLAB ----
            NSLAB = 3
            SLABG = 4          # groups per slab
            slabrt = singles.tile([96, SLABG * GRP, W], F16,
                                  name="slabrt", tag="slabrt")
            slabring = [slabrt[32 * k:32 * k + 3] for k in range(NSLAB)]
            for t_ in slabring:
                nc.vector.memset(t_[0:1, :, :], 1.0)

            # ---- staging helpers ----
            def issue_s4(g):
                qb = quad_bases[g]
                s4 = s4p.tile([128, W], F16, name=f"s4_{g}", tag="s4")
                in_ap = src[:, qb:qb + 4, :].rearrange("c r x -> r c x")
                nc.gpsimd.dma_start(out=s4, in_=in_ap)
                return s4

            # ---- prologue: source prefetches first (no deps) ----
            nslabs = (ngr + SLABG - 1) // SLABG
            pre_s4 = {}
            pre_slab = {}
            for g in range(min(5, ngr)):
                pre_s4[g] = issue_s4(g)

            # ---- phase 1: disparity -> int/frac fp16 [128, nt, W] ----
            int16 = singles.tile([128, nt, W], F16)
            frac16 = singles.tile([128, nt, W], F16)

            def phase1_tile(t):
                r0 = 128 * t
                nr = min(128, H - r0)
                dt_ = ph1.tile([128, W], F32)
                nc.sync.dma_start(out=dt_[:nr], in_=disp[r0:r0 + nr, :])
                ixm1 = ph1.tile([128, W], F32)
                nc.vector.tensor_scalar_mul(ixm1[:nr], dt_[:nr], -float(S))
                nc.vector.tensor_add(ixm1[:nr], ixm1[:nr], cfb[:nr])
                iv = ph1.tile([128, W], I32)
                nc.vector.tensor_copy(iv[:nr], ixm1[:nr])
                fv = ph1.tile([128, W], F32)
                nc.vector.tensor_copy(fv[:nr], iv[:nr])
                nc.scalar.copy(int16[:nr, t, :], fv[:nr])
                fr = ph1.tile([128, W], F32)
                nc.vector.tensor_sub(fr[:nr], ixm1[:nr], fv[:nr])
                nc.scalar.copy(frac16[:nr, t, :], fr[:nr])

            def issue_slab(sidx):
                # One slab serves groups [SLABG*sidx, SLABG*(sidx+1)).
                # slab partitions: 0 = ones (persistent), 1 = int, 2 = frac.
                y0 = SLABG * GRP * sidx
                cnt = min(SLABG * GRP, nrows - y0)
                slab = slabring[sidx % NSLAB]
                for part, tsrc in ((1, int16), (2, frac16)):
                    done = 0
                    while done < cnt:
                        y = y0 + done
                        p, t = y % 128, y // 128
                        n = min(cnt - done, 128 - p)
                        nc.sync.dma_start(
                            out=slab[part:part + 1, done:done + n, :],
                            in_=tsrc[p:p + n, t, :])
                        done += n
                return slab

            # phase-1 tile 0 unlocks the first two slabs; later tiles
            # are processed after the prologue prefetches are queued.
            phase1_tile(0)
            for p_ in range(min(2, nslabs)):
                pre_slab[p_] = issue_slab(p_)
            for t in range(1, nt):
                phase1_tile(t)

            def gathers_one(rec, i, j, outp):
                wsb = rec["wsb"][i]
                vtx = rec["vtx"]
                n = min(BLK, W - BLK * j)
                nc.tensor.matmul(
                    outp[32 * i:32 * i + 32, OCOL[j]:OCOL[j] + n],
                    vtx[:, XCOL[j] + 32 * i:XCOL[j] + 32 * i + 32],
                    wsb[:, BLK * j:BLK * j + n],
                    start=True, stop=True,
                    tile_position=(0, 32 * i),
                )

            # Output stores are batched OBATCH groups per DMA: per-group PSUM
            # evac copies land in one wide SBUF tile; a single DMA (emitted
            # right after the batch's last copy, so its wait is ~resolved)
            # stores 3*OBATCH rows.  Few out DMAs -> the 8 round-robin DMAHW
            # completion lanes are never blocked by long-waiting stores, which
            # otherwise throttles the prefetch DMA stream behind them.
            OBATCH = 8
            obuf_state = {"tile": None, "base": -1}
            pending_out = []   # queued store DMAs, drained 1-2 per iteration

            def obuf_for(pg):
                b = pg % OBATCH
                if b == 0:
                    obuf_state["tile"] = obufp.tile([96, OBATCH, W], F32,
                                                    name=f"obuf_{pg}",
                                                    tag="obuf")
                    obuf_state["base"] = pg
                return obuf_state["tile"], b

            def store_flush(prev):
                pg = prev["g"]
                b = pg % OBATCH
                if b == OBATCH - 1 or pg == ngr - 1:
                    g0 = obuf_state["base"]
                    nb = pg - g0 + 1
                    obuf = obuf_state["tile"]
                    rows = outd[:, GRP * g0:GRP * (g0 + nb), :]
                    for i in range(GRP):
                        pending_out.append(
                            (rows[:, i::GRP, :], obuf[32 * i:32 * i + 32, 0:nb, :]))

            def drain_out(all_=False):
                # 1-2 store DMAs per iteration: data is >=1 batch old, so the
                # wait is resolved at issue and the SP queue never head-blocks.
                if all_:
                    n = len(pending_out)
                else:
                    n = min(len(pending_out), 2 if len(pending_out) > 3 else 1)
                for _ in range(n):
                    o, i_ = pending_out.pop(0)
                    nc.sync.dma_start(out=o, in_=i_)

            def evac_out(rec):
                # outp evac one iteration after its gathers: both engines
                # are idle at iteration start, and the PSUM frees before
                # this iteration's own gathers need the banks.
                obuf, b = obuf_for(rec["g"])
                nc.vector.tensor_copy(obuf[:, b, 0:470], rec["outp"][:, 0:470])
                nc.scalar.copy(obuf[:, b, 470:W], rec["outp"][:, 512:810])
                store_flush(rec)

            prev = None      # group whose gathers run this iteration
            pout = None      # gathered-but-not-evacuated output record
            for g in range(ngr):
                s4 = pre_s4.pop(g)
                slab = pre_slab[g // SLABG]
                ld3sb = ld3rt[32 * ((g // SLABG) % 3):32 * ((g // SLABG) % 3) + 3]
                if g % SLABG == SLABG - 1:
                    del pre_slab[g // SLABG]
                iloc0 = GRP * (g % SLABG)

                # ---- prefetch first: these SP DMAs have no unresolved waits,
                # so they must sit AHEAD of the out DMAs in the SP queue ----
                if g + 5 < ngr:
                    pre_s4[g + 5] = issue_s4(g + 5)
                if g % SLABG == 0 and g // SLABG + 2 < nslabs:
                    pre_slab[g // SLABG + 2] = issue_slab(g // SLABG + 2)
                drain_out()

                if pout is not None:
                    evac_out(pout)
                    pout = None

                # ---- D rows 0,1 FIRST: unblocks the ACT/DVE weight chain
                # at iteration start so it runs under the PE stream ----
                dp0 = dpp.tile([128, 1024], F32, name=f"dp0_{g}", tag="dp")
                nc.tensor.matmul(dp0[:, 0:512], ld3sb,
                                 slab[0:3, iloc0 + 0, 0:512],
                                 start=True, stop=True)
                nc.tensor.matmul(dp0[:, 512:768], ld3sb,
                                 slab[0:3, iloc0 + 0, 512:W],
                                 start=True, stop=True)
                dp1 = dpp.tile([128, 1024], F32, name=f"dp1_{g}", tag="dp")
                nc.tensor.matmul(dp1[:, 0:512], ld3sb,
                                 slab[0:3, iloc0 + 1, 0:512],
                                 start=True, stop=True)
                nc.tensor.matmul(dp1[:, 512:768], ld3sb,
                                 slab[0:3, iloc0 + 1, 512:W],
                                 start=True, stop=True)

                asb0 = asbp.tile([128, W], F16, name=f"asb0_{g}", tag="asb")
                nc.scalar.activation(asb0, dp0[:, 0:W], AF.Abs)
                wsb0 = wp.tile([128, W], F16, name=f"wsb0_{g}", tag="wsb")
                nc.vector.tensor_scalar(out=wsb0, in0=asb0, scalar1=1.0,
                                        scalar2=0.0, op0=ALU.subtract,
                                        op1=ALU.min)
                asb1 = asbp.tile([128, W], F16, name=f"asb1_{g}", tag="asb")
                nc.scalar.activation(asb1, dp1[:, 0:W], AF.Abs)
                wsb1 = wp.tile([128, W], F16, name=f"wsb1_{g}", tag="wsb")
                nc.vector.tensor_scalar(out=wsb1, in0=asb1, scalar1=1.0,
                                        scalar2=0.0, op0=ALU.subtract,
                                        op1=ALU.min)

                # ---- gathers (prev) j0-4 while the weight chain runs ----
                outp = None
                if prev is not None:
                    outp = outpp.tile([96, 1024], F32, name=f"outp_{g}",
                                      tag="outp")
                    for j_ in range(5):
                        for i_ in range(GRP):
                            gathers_one(prev, i_, j_, outp)

                # ---- D row 2 + chain (early: its wsb gates the next
                # iteration's first gathers) ----
                dp2 = dpp.tile([128, 1024], F32, name=f"dp2_{g}", tag="dp")
                nc.tensor.matmul(dp2[:, 0:512], ld3sb,
                                 slab[0:3, iloc0 + 2, 0:512],
                                 start=True, stop=True)
                nc.tensor.matmul(dp2[:, 512:768], ld3sb,
                                 slab[0:3, iloc0 + 2, 512:W],
                                 start=True, stop=True)
                asb2 = asbp.tile([128, W], F16, name=f"asb2_{g}", tag="asb")
                nc.scalar.activation(asb2, dp2[:, 0:W], AF.Abs)
                wsb2 = wp.tile([128, W], F16, name=f"wsb2_{g}", tag="wsb")
                nc.vector.tensor_scalar(out=wsb2, in0=asb2, scalar1=1.0,
                                        scalar2=0.0, op0=ALU.subtract,
                                        op1=ALU.min)

                # ---- VT: blend-transpose, 9 full-width windows ----
                vtp = vtpp.tile([128, 1024], F32, name=f"vtp_{g}", tag="vtp")
                for j in range(NB):
                    nc.tensor.matmul(
                        vtp[:, VCOL[j]:VCOL[j] + GRP * C],
                        s4[:, WIN_LO[j]:WIN_HI[j]],
                        v2sb[:, g, :],
                        start=True, stop=True,
                    )
                vtx = vtxp.tile([128, 864], F16, name=f"vtx_{g}", tag="vtx")
                nc.vector.tensor_copy(vtx[:, 0:480], vtp[:, 0:480])
                nc.vector.tensor_copy(vtx[:, 480:864], vtp[:, 512:896])

                # ---- gathers (prev) j5-8 ----
                if prev is not None:
                    for j_ in range(5, NB):
                        for i_ in range(GRP):
                            gathers_one(prev, i_, j_, outp)
                    pout = {"g": prev["g"], "outp": outp}

                prev = {"g": g, "wsb": (wsb0, wsb1, wsb2), "vtx": vtx}

            # ---- epilogue: evac pending, then drain last group ----
            if pout is not None:
                evac_out(pout)
            outp = outpp.tile([96, 1024], F32, name="outp_last", tag="outp")
            for j_ in range(NB):
                for i_ in range(GRP):
                    gathers_one(prev, i_, j_, outp)
            evac_out({"g": prev["g"], "outp": outp})
            drain_out(all_=True)

        if repeat > 1:
            with tc.For_i(0, repeat):
                emit_body()
        else:
            emit_body()


    nc.finalize()
    return nc


_NC_CACHE = {}


def _get_nc(ngroups=NG):
    if ngroups not in _NC_CACHE:
        _NC_CACHE[ngroups] = build_nc(ngroups)
    return _NC_CACHE[ngroups]


# ---------------------------------------------------------------- entry
def kernel(src: np.ndarray, disparity: np.ndarray) -> np.ndarray:
    from concourse.bass_utils import run_bass_kernel_spmd

    src = np.ascontiguousarray(np.asarray(src), dtype=np.float32)
    disparity = np.ascontiguousarray(np.asarray(disparity), dtype=np.float32)
    v2neg, ld3, onesw, cf, _ = _host_constants()
    nc = _get_nc()
    in_maps = []
    for b in range(B):
        in_maps.append({
            "src": src[b],
            "disp": disparity[b, 0],
            "v2neg": v2neg,
            "ld3": ld3,
            "onesw": onesw,
            "cf": cf,
        })
    res = run_bass_kernel_spmd(nc, in_maps, core_ids=list(range(N_CORES)))
    out = np.stack([res.results[b]["out"] for b in range(B)])
    return out.astype(np.float32)


# ---------------------------------------------------------------- sim test
def _np_reference(src, disp):
    """Single-core numpy reference (mirror of reference.py)."""
    Cc, Hh, Ww = src.shape
    xx = np.arange(Ww, dtype=np.float32)
    ix = (xx[None, :] - disp) * (Ww / (Ww - 1)) - 0.5          # [H, W]
    yy = np.arange(Hh, dtype=np.float32)
    iy = np.broadcast_to((yy * (Hh / (Hh - 1)) - 0.5)[:, None], (Hh, Ww))
    x0 = np.floor(ix).astype(np.int64)
    y0 = np.floor(iy).astype(np.int64)
    fx = ix - x0
    fy = iy - y0

    def gather(yi, xi):
        inb = ((yi >= 0) & (yi < Hh) & (xi >= 0) & (xi < Ww))
        yc = np.clip(yi, 0, Hh - 1)
        xc = np.clip(xi, 0, Ww - 1)
        v = src[:, yc, xc]                                      # [C, H, W]
        return v * inb[None]

    w00 = (1 - fy) * (1 - fx)
    w01 = (1 - fy) * fx
    w10 = fy * (1 - fx)
    w11 = fy * fx
    return (gather(y0, x0) * w00 + gather(y0, x0 + 1) * w01 +
            gather(y0 + 1, x0) * w10 + gather(y0 + 1, x0 + 1) * w11)


def _sim_check(ngroups=2):
    from concourse.bass_interp import CoreSim

    rng = np.random.default_rng(0)
    src = rng.standard_normal((C, H, W)).astype(np.float32)
    disp = (rng.random((H, W)) * 32.0).astype(np.float32)
    v2neg, ld3, onesw, cf, _ = _host_constants()

    nc = build_nc(ngroups)
    sim = CoreSim(nc)
    for name, val in (("src", src), ("disp", disp), ("v2neg", v2neg),
                      ("ld3", ld3), ("onesw", onesw), ("cf", cf)):
        sim.tensor(name)[:] = val
    sim.simulate(check_with_hw=False)
    got = np.array(sim.tensor("out"))

    ref = _np_reference(src, disp)
    ys = slice(0, GRP * ngroups)
    diff = got[:, ys] - ref[:, ys]
    rel = np.linalg.norm(diff) / np.linalg.norm(ref[:, ys])
    print(f"sim rows[0:{GRP * ngroups}]  max abs "
          f"{np.abs(diff).max():.3e}  rel l2 {rel:.3e}")
    return rel


if __name__ == "__main__":
    ng = int(sys.argv[1]) if len(sys.argv) > 1 else 2
    _sim_check(ng)


# revision 14
# speedup vs baseline: 1.4553x; 1.3050x over previous
"""DisparityWarp Trainium2 kernel (Bass/Tile) — v3.

Contract: kernel(src, disparity) takes FULL inputs
  src [8, 32, 384, 768] f32, disparity [8, 1, 384, 768] f32
and returns the FULL output [8, 32, 384, 768] f32 of
  grid_sample(src, grid, bilinear, zeros, align_corners=False)
with grid_x = 2*(xx - disp)/(W-1) - 1, grid_y = 2*yy/(H-1) - 1.

Sharding: pure data parallel, one batch per NeuronCore (8 cores).

Per-core algorithm: unnormalized coords ix = (x - d)*W/(W-1) - 0.5,
iy = y*H/(H-1) - 0.5. Vertical lerp weights depend only on y; the
horizontal warp is banded: out[c,x] = sum_x' vrow[c,x'] * hat(ix[x]-x')
with hat(u) = max(0, 1-|u|), and x' restricted to [0, W) (zero pad).

Geometry: output cols in blocks of BLK=94; window j covers
x' in [lo_j, hi_j), lo_j = max(94j-33, 0), hi_j = min(94j+95, W).

Per 3-row group g (quad base qb = clamp(floor(iy[3g]), 0, H-4)):
  s4   [128=(4r,32c), W] f16  <- gpsimd casting DMA of src[:, qb:qb+4, :]
  D:   dp_i = ld3.T @ [ones;int;frac] = ixm1[x] - (p+1), emitted FIRST
       so the evac engines start early.
  VT:  vtp[x'loc, (3i,32c)] = s4[:, lo_j:hi_j].T @ v2neg[g]  (9 matmuls)
  vtx  [128, 864] f16 <- vtp (ACT takes cols 0:480, DVE the rest)
  weight chain per row i: asb = |dp_i| computed in TWO engine-parallel
       halves (ACT cols 0:384, DVE 384:768; DVE uses abs_max) so the
       dp PSUM buffer frees at max(engine) latency, not their sum;
       wsb_i = min(asb-1, 0) == -hat, one DVE 4x f16 op per row.
  gathers: outp[32i:+32, blk_j] = vtx[.., win_j, i].T @ wsb_i[:, blk_j]
       (tile_position (0,32i); window 8 contracts K=49 only)
  outp evac is split by PSUM bank: bank0 (blocks 0-4) evacs on DVE as
       soon as j=4's gathers land; bank1 (blocks 5-8) on ACT after j=8.

The PE stream runs gathers one group behind VT/D so the weight chain
(ACT/DVE) hides behind the next group's PE work.  DMA discipline (the
real bottleneck on this stack): prefetch DMAs (s4 via Pool/SWDGE with
f32->f16 cast; slab staging via SP) are issued 4-16 groups ahead and
never wait; output stores are batched 8 groups per DMA set, buffered
through a wide SBUF tile, and drained 1-2 per iteration with a
backlog so their waits are resolved at issue.
"""

import sys

if "/opt/trn_rl_repo" not in sys.path:
    sys.path.insert(0, "/opt/trn_rl_repo")

from contextlib import ExitStack

import numpy as np

import concourse.bass as bass
import concourse.mybir as mybir
from concourse import bacc
from concourse.tile import TileContext

F32 = mybir.dt.float32
F16 = mybir.dt.float16
I32 = mybir.dt.int32
AF = mybir.ActivationFunctionType
ALU = mybir.AluOpType

B, C, H, W = 8, 32, 384, 768
S = W / (W - 1)
BLK = 94           # output columns per block
NB = 9             # ceil(W / BLK)
GRP = 3            # output rows per group
NG = H // GRP      # 128 groups
N_CORES = 8

WIN_LO = [max(min(BLK * j - 33, W - 128), 0) for j in range(NB)]
WIN_HI = [lo + 128 for lo in WIN_LO]
# vtp / outp PSUM column offsets (bank-aligned: banks of 512 f32)
VCOL = [96 * j if j < 5 else 512 + 96 * (j - 5) for j in range(NB)]
OCOL = [BLK * j if j < 5 else 512 + BLK * (j - 5) for j in range(NB)]
# merged vt SBUF tile column offsets (no bank constraint in SBUF)
XCOL = [96 * j for j in range(NB)]
HALF = 384         # asb engine-split point


# ---------------------------------------------------------------- constants
def _vert_coefs():
    yy = np.arange(H, dtype=np.float64)
    iy = yy * (H / (H - 1)) - 0.5
    y0 = np.floor(iy).astype(np.int64)
    fy = iy - y0
    a = (1.0 - fy) * ((y0 >= 0) & (y0 < H))
    b = fy * ((y0 + 1 >= 0) & (y0 + 1 < H))
    return a, b, y0


def _host_constants():
    a, b, y0 = _vert_coefs()
    # V2NEG [128 p=(4r,32c), NG, 96 m=(3i,32c)] fp16, negated blend coefs
    v2 = np.zeros((4, C, NG, GRP, C), dtype=np.float32)
    quad_bases = []
    for g in range(NG):
        ys = [GRP * g + i for i in range(GRP)]
        qbase = min(max(int(y0[ys[0]]), 0), H - 4)
        quad_bases.append(qbase)
        for i, y in enumerate(ys):
            ra = int(y0[y]) - qbase
            rb = ra + 1
            for c in range(C):
                if a[y] != 0.0:
                    assert 0 <= ra <= 3
                    v2[ra, c, g, i, c] += -a[y]
                if b[y] != 0.0:
                    assert 0 <= rb <= 3
                    v2[rb, c, g, i, c] += -b[y]
    v2neg = v2.reshape(4 * C, NG, GRP * C).astype(np.float16)

    # D matmul stationary: rows [-(p+1), 1, 1]; rhs rows [ones, int, frac]
    ld3 = np.stack([
        -(np.arange(128, dtype=np.float32) + 1.0),
        np.ones(128, dtype=np.float32),
        np.ones(128, dtype=np.float32),
    ]).astype(np.float16)                                        # [3, 128]
    onesw = np.ones((1, 2 * GRP, W), dtype=np.float16)           # [1, 6, W]

    x = np.arange(W, dtype=np.float64)
    base = np.array([WIN_LO[int(xi) // BLK] for xi in x], dtype=np.float64)
    cf = (x * S - 0.5 - base + 1.0).astype(np.float32)[None, :]  # [1, W]
    return v2neg, ld3, onesw, cf, quad_bases


# ---------------------------------------------------------------- program
def build_nc(ngroups=NG, repeat=1):
    _, _, _, _, quad_bases = _host_constants()  # (v2neg, ld3, onesw, cf, qb)
    nc = bacc.Bacc("TRN2", target_bir_lowering=False, debug=False)

    src = nc.dram_tensor("src", [C, H, W], F32, kind="ExternalInput").ap()
    disp = nc.dram_tensor("disp", [H, W], F32, kind="ExternalInput").ap()
    v2d = nc.dram_tensor("v2neg", [4 * C, NG, GRP * C], F16,
                         kind="ExternalInput").ap()
    ld3d = nc.dram_tensor("ld3", [3, 128], F16, kind="ExternalInput").ap()
    onesd = nc.dram_tensor("onesw", [1, 2 * GRP, W], F16,
                           kind="ExternalInput").ap()
    cfd = nc.dram_tensor("cf", [1, W], F32, kind="ExternalInput").ap()
    outd = nc.dram_tensor("out", [C, H, W], F32, kind="ExternalOutput").ap()

    ngr = min(ngroups, NG)
    nrows = GRP * ngr
    nt = (nrows + 127) // 128

    with ExitStack() as ctx:
        tc = ctx.enter_context(TileContext(nc))
        singles = ctx.enter_context(tc.tile_pool(name="singles", bufs=1))
        ph1 = ctx.enter_context(tc.tile_pool(name="ph1", bufs=1))
        s4p = ctx.enter_context(tc.tile_pool(name="s4p", bufs=9))
        vtxp = ctx.enter_context(tc.tile_pool(name="vtxp", bufs=3))
        asbp = ctx.enter_context(tc.tile_pool(name="asbp", bufs=3))
        wp = ctx.enter_context(tc.tile_pool(name="wp", bufs=6))
        obufp = ctx.enter_context(tc.tile_pool(name="obufp", bufs=2))
        vtpp = ctx.enter_context(tc.tile_pool(name="vtpp", bufs=1, space="PSUM"))
        dpp = ctx.enter_context(tc.tile_pool(name="dpp", bufs=2, space="PSUM"))
        outpp = ctx.enter_context(tc.tile_pool(name="outpp", bufs=1, space="PSUM"))

        def emit_body():
            # ---- constants ----
            v2sb = singles.tile([4 * C, NG, GRP * C], F16)
            nc.sync.dma_start(out=v2sb, in_=v2d)
            # ld3 replicated at bases 0/32/64 to match the slab ring's
            # base partition (matmul operands must share base_partition)
            ld3rt = singles.tile([96, 128], F16, name="ld3rt", tag="ld3rt")
            for k in range(3):
                nc.sync.dma_start(out=ld3rt[32 * k:32 * k + 3], in_=ld3d)
            cfb = singles.tile([128, W], F32)
            nc.sync.dma_start(out=cfb, in_=cfd.to_broadcast((128, W)))

            # ---- persistent slab ring, packed on partitions (3 per
            # entry) so the ring costs one column range, not NSLAB ----
            NSLAB = 3
            SLABG = 4          # groups per slab
            slabrt = singles.tile([96, SLABG * GRP, W], F16,
                                  name="slabrt", tag="slabrt")
            slabring = [slabrt[32 * k:32 * k + 3] for k in range(NSLAB)]
            for t_ in slabring:
                nc.vector.memset(t_[0:1, :, :], 1.0)

            # ---- staging helpers ----
            def issue_s4(g):
                qb = quad_bases[g]
                s4 = s4p.tile([128, W], F16, name=f"s4_{g}", tag="s4")
                in_ap = src[:, qb:qb + 4, :].rearrange("c r x -> r c x")
                nc.gpsimd.dma_start(out=s4, in_=in_ap)
                return s4

            # ---- prologue: source prefetches first (no deps) ----
            nslabs = (ngr + SLABG - 1) // SLABG
            pre_s4 = {}
            pre_slab = {}
            for g in range(min(5, ngr)):
                pre_s4[g] = issue_s4(g)

            # ---- phase 1: disparity -> int/frac fp16 [128, nt, W] ----
            int16 = singles.tile([128, nt, W], F16)
            frac16 = singles.tile([128, nt, W], F16)

            def phase1_tile(t):
                r0 = 128 * t
                nr = min(128, H - r0)
                dt_ = ph1.tile([128, W], F32)
                nc.sync.dma_start(out=dt_[:nr], in_=disp[r0:r0 + nr, :])
                ixm1 = ph1.tile([128, W], F32)
                nc.vector.tensor_scalar_mul(ixm1[:nr], dt_[:nr], -float(S))
                nc.vector.tensor_add(ixm1[:nr], ixm1[:nr], cfb[:nr])
                iv = ph1.tile([128, W], I32)
                nc.vector.tensor_copy(iv[:nr], ixm1[:nr])
                fv = ph1.tile([128, W], F32)
                nc.vector.tensor_copy(fv[:nr], iv[:nr])
                nc.scalar.copy(int16[:nr, t, :], fv[:nr])
                fr = ph1.tile([128, W], F32)
                nc.vector.tensor_sub(fr[:nr], ixm1[:nr], fv[:nr])
                nc.scalar.copy(frac16[:nr, t, :], fr[:nr])

            def issue_slab(sidx):
                # One slab serves groups [SLABG*sidx, SLABG*(sidx+1)).
                # slab partitions: 0 = ones (persistent), 1 = int, 2 = frac.
                y0 = SLABG * GRP * sidx
                cnt = min(SLABG * GRP, nrows - y0)
                slab = slabring[sidx % NSLAB]
                for part, tsrc in ((1, int16), (2, frac16)):
                    done = 0
                    while done < cnt:
                        y = y0 + done
                        p, t = y % 128, y // 128
                        n = min(cnt - done, 128 - p)
                        nc.sync.dma_start(
                            out=slab[part:part + 1, done:done + n, :],
                            in_=tsrc[p:p + n, t, :])
                        done += n
                return slab

            # phase-1 tile 0 unlocks the first two slabs; later tiles
            # are processed after the prologue prefetches are queued.
            phase1_tile(0)
            for p_ in range(min(2, nslabs)):
                pre_slab[p_] = issue_slab(p_)
            for t in range(1, nt):
                phase1_tile(t)

            def gathers_one(rec, i, j, outp):
                wsb = rec["wsb"][i]
                vtx = rec["vtx"]
                n = min(BLK, W - BLK * j)
                nc.tensor.matmul(
                    outp[32 * i:32 * i + 32, OCOL[j]:OCOL[j] + n],
                    vtx[:, XCOL[j] + 32 * i:XCOL[j] + 32 * i + 32],
                    wsb[:, BLK * j:BLK * j + n],
                    start=True, stop=True,
                    tile_position=(0, 32 * i),
                )

            # Output stores are batched OBATCH groups per DMA: per-group PSUM
            # evac copies land in one wide SBUF tile; a single DMA (emitted
            # right after the batch's last copy, so its wait is ~resolved)
            # stores 3*OBATCH rows.  Few out DMAs -> the 8 round-robin DMAHW
            # completion lanes are never blocked by long-waiting stores, which
            # otherwise throttles the prefetch DMA stream behind them.
            OBATCH = 8
            obuf_state = {"tile": None, "base": -1}
            pending_out = []   # queued store DMAs, drained 1-2 per iteration

            def obuf_for(pg):
                b = pg % OBATCH
                if b == 0:
                    obuf_state["tile"] = obufp.tile([96, OBATCH, W], F32,
                                                    name=f"obuf_{pg}",
                                                    tag="obuf")
                    obuf_state["base"] = pg
                return obuf_state["tile"], b

            def store_flush(prev):
                pg = prev["g"]
                b = pg % OBATCH
                if b == OBATCH - 1 or pg == ngr - 1:
                    g0 = obuf_state["base"]
                    nb = pg - g0 + 1
                    obuf = obuf_state["tile"]
                    rows = outd[:, GRP * g0:GRP * (g0 + nb), :]
                    for i in range(GRP):
                        pending_out.append(
                            (rows[:, i::GRP, :], obuf[32 * i:32 * i + 32, 0:nb, :]))

            def drain_out(all_=False):
                # 1-2 store DMAs per iteration: data is >=1 batch old, so the
                # wait is resolved at issue and the SP queue never head-blocks.
                if all_:
                    n = len(pending_out)
                else:
                    n = min(len(pending_out), 2 if len(pending_out) > 3 else 1)
                for _ in range(n):
                    o, i_ = pending_out.pop(0)
                    nc.sync.dma_start(out=o, in_=i_)

            ob1_pend = []

            def evac_out(rec):
                # outp evac one iteration after its gathers: bank0 on DVE at
                # iteration start (frees PSUM for this iteration's j0-4);
                # bank1 on ACT but deferred past the abs chain (it only gates
                # j5-8, and ACT's asb latency feeds the dp rotation).
                obuf, b = obuf_for(rec["g"])
                nc.vector.tensor_copy(obuf[:, b, 0:470], rec["outp"][:, 0:470])
                ob1_pend.append((obuf, b, rec))

            def evac_out_flush():
                while ob1_pend:
                    ob_, b_, rec_ = ob1_pend.pop(0)
                    nc.scalar.copy(ob_[:, b_, 470:W], rec_["outp"][:, 512:810])
                    store_flush(rec_)

            prev = None      # group whose gathers run this iteration
            pout = None      # gathered-but-not-evacuated output record
            for g in range(ngr):
                s4 = pre_s4.pop(g)
                slab = pre_slab[g // SLABG]
                ld3sb = ld3rt[32 * ((g // SLABG) % 3):32 * ((g // SLABG) % 3) + 3]
                if g % SLABG == SLABG - 1:
                    del pre_slab[g // SLABG]
                iloc0 = GRP * (g % SLABG)

                # ---- prefetch first: these SP DMAs have no unresolved waits,
                # so they must sit AHEAD of the out DMAs in the SP queue ----
                if g + 5 < ngr:
                    pre_s4[g + 5] = issue_s4(g + 5)
                if g % SLABG == 0 and g // SLABG + 2 < nslabs:
                    pre_slab[g // SLABG + 2] = issue_slab(g // SLABG + 2)
                drain_out()

                if pout is not None:
                    evac_out(pout)
                    pout = None

                # ---- D rows 0,1 FIRST: unblocks the ACT/DVE weight chain
                # at iteration start so it runs under the PE stream ----
                dp0 = dpp.tile([128, 1024], F32, name=f"dp0_{g}", tag="dp")
                nc.tensor.matmul(dp0[:, 0:512], ld3sb,
                                 slab[0:3, iloc0 + 0, 0:512],
                                 start=True, stop=True)
                nc.tensor.matmul(dp0[:, 512:768], ld3sb,
                                 slab[0:3, iloc0 + 0, 512:W],
                                 start=True, stop=True)
                dp1 = dpp.tile([128, 1024], F32, name=f"dp1_{g}", tag="dp")
                nc.tensor.matmul(dp1[:, 0:512], ld3sb,
                                 slab[0:3, iloc0 + 1, 0:512],
                                 start=True, stop=True)
                nc.tensor.matmul(dp1[:, 512:768], ld3sb,
                                 slab[0:3, iloc0 + 1, 512:W],
                                 start=True, stop=True)

                asb0 = asbp.tile([128, W], F16, name=f"asb0_{g}", tag="asb")
                nc.scalar.activation(asb0, dp0[:, 0:W], AF.Abs)
                wsb0 = wp.tile([128, W], F16, name=f"wsb0_{g}", tag="wsb")
                nc.vector.tensor_scalar(out=wsb0, in0=asb0, scalar1=1.0,
                                        scalar2=0.0, op0=ALU.subtract,
                                        op1=ALU.min)
                asb1 = asbp.tile([128, W], F16, name=f"asb1_{g}", tag="asb")
                nc.scalar.activation(asb1, dp1[:, 0:W], AF.Abs)
                wsb1 = wp.tile([128, W], F16, name=f"wsb1_{g}", tag="wsb")
                nc.vector.tensor_scalar(out=wsb1, in0=asb1, scalar1=1.0,
                                        scalar2=0.0, op0=ALU.subtract,
                                        op1=ALU.min)

                # ---- gathers (prev) j0-4 while the weight chain runs ----
                outp = None
                if prev is not None:
                    outp = outpp.tile([96, 1024], F32, name=f"outp_{g}",
                                      tag="outp")
                    for j_ in range(5):
                        for i_ in range(GRP):
                            gathers_one(prev, i_, j_, outp)

                # ---- D row 2 + chain (early: its wsb gates the next
                # iteration's first gathers) ----
                dp2 = dpp.tile([128, 1024], F32, name=f"dp2_{g}", tag="dp")
                nc.tensor.matmul(dp2[:, 0:512], ld3sb,
                                 slab[0:3, iloc0 + 2, 0:512],
                                 start=True, stop=True)
                nc.tensor.matmul(dp2[:, 512:768], ld3sb,
                                 slab[0:3, iloc0 + 2, 512:W],
                                 start=True, stop=True)
                asb2 = asbp.tile([128, W], F16, name=f"asb2_{g}", tag="asb")
                nc.scalar.activation(asb2, dp2[:, 0:W], AF.Abs)
                wsb2 = wp.tile([128, W], F16, name=f"wsb2_{g}", tag="wsb")
                nc.vector.tensor_scalar(out=wsb2, in0=asb2, scalar1=1.0,
                                        scalar2=0.0, op0=ALU.subtract,
                                        op1=ALU.min)

                # ---- VT: blend-transpose, 9 full-width windows ----
                vtp = vtpp.tile([128, 1024], F32, name=f"vtp_{g}", tag="vtp")
                for j in range(NB):
                    nc.tensor.matmul(
                        vtp[:, VCOL[j]:VCOL[j] + GRP * C],
                        s4[:, WIN_LO[j]:WIN_HI[j]],
                        v2sb[:, g, :],
                        start=True, stop=True,
                    )
                vtx = vtxp.tile([128, 864], F16, name=f"vtx_{g}", tag="vtx")
                nc.vector.tensor_copy(vtx[:, 0:480], vtp[:, 0:480])
                nc.vector.tensor_copy(vtx[:, 480:864], vtp[:, 512:896])

                evac_out_flush()

                # ---- gathers (prev) j5-8 ----
                if prev is not None:
                    for j_ in range(5, NB):
                        for i_ in range(GRP):
                            gathers_one(prev, i_, j_, outp)
                    pout = {"g": prev["g"], "outp": outp}

                prev = {"g": g, "wsb": (wsb0, wsb1, wsb2), "vtx": vtx}

            # ---- epilogue: evac pending, then drain last group ----
            if pout is not None:
                evac_out(pout)
                evac_out_flush()
            outp = outpp.tile([96, 1024], F32, name="outp_last", tag="outp")
            for j_ in range(NB):
                for i_ in range(GRP):
                    gathers_one(prev, i_, j_, outp)
            evac_out({"g": prev["g"], "outp": outp})
            evac_out_flush()
            drain_out(all_=True)

        if repeat > 1:
            with tc.For_i(0, repeat):
                emit_body()
        else:
            emit_body()


    nc.finalize()
    return nc


_NC_CACHE = {}


def _get_nc(ngroups=NG):
    if ngroups not in _NC_CACHE:
        _NC_CACHE[ngroups] = build_nc(ngroups)
    return _NC_CACHE[ngroups]


# ---------------------------------------------------------------- entry
def kernel(src: np.ndarray, disparity: np.ndarray) -> np.ndarray:
    from concourse.bass_utils import run_bass_kernel_spmd

    src = np.ascontiguousarray(np.asarray(src), dtype=np.float32)
    disparity = np.ascontiguousarray(np.asarray(disparity), dtype=np.float32)
    v2neg, ld3, onesw, cf, _ = _host_constants()
    nc = _get_nc()
    in_maps = []
    for b in range(B):
        in_maps.append({
            "src": src[b],
            "disp": disparity[b, 0],
            "v2neg": v2neg,
            "ld3": ld3,
            "onesw": onesw,
            "cf": cf,
        })
    res = run_bass_kernel_spmd(nc, in_maps, core_ids=list(range(N_CORES)))
    out = np.stack([res.results[b]["out"] for b in range(B)])
    return out.astype(np.float32)


# ---------------------------------------------------------------- sim test
def _np_reference(src, disp):
    """Single-core numpy reference (mirror of reference.py)."""
    Cc, Hh, Ww = src.shape
    xx = np.arange(Ww, dtype=np.float32)
    ix = (xx[None, :] - disp) * (Ww / (Ww - 1)) - 0.5          # [H, W]
    yy = np.arange(Hh, dtype=np.float32)
    iy = np.broadcast_to((yy * (Hh / (Hh - 1)) - 0.5)[:, None], (Hh, Ww))
    x0 = np.floor(ix).astype(np.int64)
    y0 = np.floor(iy).astype(np.int64)
    fx = ix - x0
    fy = iy - y0

    def gather(yi, xi):
        inb = ((yi >= 0) & (yi < Hh) & (xi >= 0) & (xi < Ww))
        yc = np.clip(yi, 0, Hh - 1)
        xc = np.clip(xi, 0, Ww - 1)
        v = src[:, yc, xc]                                      # [C, H, W]
        return v * inb[None]

    w00 = (1 - fy) * (1 - fx)
    w01 = (1 - fy) * fx
    w10 = fy * (1 - fx)
    w11 = fy * fx
    return (gather(y0, x0) * w00 + gather(y0, x0 + 1) * w01 +
            gather(y0 + 1, x0) * w10 + gather(y0 + 1, x0 + 1) * w11)


def _sim_check(ngroups=2):
    from concourse.bass_interp import CoreSim

    rng = np.random.default_rng(0)
    src = rng.standard_normal((C, H, W)).astype(np.float32)
    disp = (rng.random((H, W)) * 32.0).astype(np.float32)
    v2neg, ld3, onesw, cf, _ = _host_constants()

    nc = build_nc(ngroups)
    sim = CoreSim(nc)
    for name, val in (("src", src), ("disp", disp), ("v2neg", v2neg),
                      ("ld3", ld3), ("onesw", onesw), ("cf", cf)):
        sim.tensor(name)[:] = val
    sim.simulate(check_with_hw=False)
    got = np.array(sim.tensor("out"))

    ref = _np_reference(src, disp)
    ys = slice(0, GRP * ngroups)
    diff = got[:, ys] - ref[:, ys]
    rel = np.linalg.norm(diff) / np.linalg.norm(ref[:, ys])
    print(f"sim rows[0:{GRP * ngroups}]  max abs "
          f"{np.abs(diff).max():.3e}  rel l2 {rel:.3e}")
    return rel


if __name__ == "__main__":
    ng = int(sys.argv[1]) if len(sys.argv) > 1 else 2
    _sim_check(ng)
